# revision 1
# baseline (speedup 1.0000x reference)
"""Multi-head causal attention (B=2, S=2048, H=16, D=64) on 8 TRN2 NeuronCores.

Sharding: data-parallel over batch (2) x tensor-parallel over head groups (4).
Core c handles batch b = c // 4, head group g = c % 4 (heads 4g..4g+3).
Each core computes q/k/v projections for its 4 heads, RoPE, causal
flash-style attention (upper-triangular blocks skipped), and a partial
output projection out_partial = attn_out @ Wo[256g:256g+256].  The host
sums the 4 partials per batch and adds the (bias) terms.

On-chip layout highlights:
 - All matmuls run as float32r (full f32 storage; TF32-class PE speed).
 - q/k are computed TRANSPOSED (d on partitions) directly by using the
   weight matrix as the stationary operand, so no on-chip transposes of x
   are needed (the host pre-transposes x once).
 - Wq/Wk columns are permuted to [all even comps | all odd comps] so RoPE
   runs as full-128-partition DVE ops; a pair of 0/1 permutation matmuls
   regroups the rotated q/k into head-contiguous layout for the scores.
 - scores are computed transposed (sk on partitions, sq free) so that the
   PV matmul consumes exp(scores) directly as the moving operand, with a
   ones-column appended to v producing the softmax denominator for free.
 - softmax runs without max-subtraction (scores are O(5) here; exp of the
   -1e9 mask underflows to exactly 0), so only the diagonal 128x128
   mask blocks are ever touched.
"""

import os
import numpy as np
from contextlib import ExitStack

import concourse.bass as bass
import concourse.tile as tile
from concourse import bacc, mybir
from concourse.alu_op_type import AluOpType
from concourse.bass_utils import run_bass_kernel_spmd

F32 = mybir.dt.float32
F32R = mybir.dt.float32r
AF = mybir.ActivationFunctionType

B, S, H, D = 2, 2048, 16, 64
HID = H * D           # 1024
NCORES = 8
G = 4                 # head groups
HPG = H // G          # heads per group = 4
DG = HPG * D          # per-group model dim = 256
KS = HID // 128       # 8 k-subtiles
NQ = 4                # S quarters (chunks of 512)
SB = S // 128         # 16 s-blocks


PA_BUFS = 2     # projection/out-proj psum slots
PSC_BUFS = 2    # paired score psum tiles (2 banks each)
PPV_BUFS = 2    # PV accumulator banks
EX_BUFS = 2     # exp staging tiles


def build_program():
    nc = bacc.Bacc("TRN2", target_bir_lowering=False, debug=False,
                   num_devices=NCORES)

    xT = nc.dram_tensor("xT", [HID, S], F32R, kind="ExternalInput").ap()
    wq = nc.dram_tensor("wq", [HID, DG], F32R, kind="ExternalInput").ap()
    wk = nc.dram_tensor("wk", [HID, DG], F32R, kind="ExternalInput").ap()
    wv = nc.dram_tensor("wv", [HID, DG], F32R, kind="ExternalInput").ap()
    wo = nc.dram_tensor("wo", [DG, HID], F32R, kind="ExternalInput").ap()
    bqp = nc.dram_tensor("bqp", [128, 2], F32, kind="ExternalInput").ap()
    bkp = nc.dram_tensor("bkp", [128, 2], F32, kind="ExternalInput").ap()
    cos4 = nc.dram_tensor("cos4", [128, S], F32, kind="ExternalInput").ap()
    sin4 = nc.dram_tensor("sin4", [128, S], F32, kind="ExternalInput").ap()
    maskT = nc.dram_tensor("maskT", [128, 256], F32, kind="ExternalInput").ap()
    permd = nc.dram_tensor("permd", [128, 4, 128], F32R, kind="ExternalInput").ap()
    onesd = nc.dram_tensor("onesd", [128, SB * HPG], F32R, kind="ExternalInput").ap()
    ones2d = nc.dram_tensor("ones2d", [33, 128], F32R, kind="ExternalInput").ap()
    out = nc.dram_tensor("out", [S, HID], F32, kind="ExternalOutput").ap()

    with tile.TileContext(nc) as tc, ExitStack() as ctx:
        const = ctx.enter_context(tc.tile_pool(name="const", bufs=1))
        xp = ctx.enter_context(tc.tile_pool(name="xp", bufs=2))
        tmp = ctx.enter_context(tc.tile_pool(name="tmp", bufs=2))
        ex = ctx.enter_context(tc.tile_pool(name="ex", bufs=EX_BUFS))
        stg = ctx.enter_context(tc.tile_pool(name="stg", bufs=2))
        nrm = ctx.enter_context(tc.tile_pool(name="nrm", bufs=2))
        pvc_pool = ctx.enter_context(tc.tile_pool(name="pvc", bufs=2))
        ps = ctx.enter_context(tc.tile_pool(name="ps", bufs=PA_BUFS, space="PSUM"))
        psc = ctx.enter_context(tc.tile_pool(name="psc", bufs=PSC_BUFS, space="PSUM"))
        ppv = ctx.enter_context(tc.tile_pool(name="ppv", bufs=PPV_BUFS, space="PSUM"))

        # ---- persistent SBUF tiles (DMAs emitted at first-use points) ----
        wq_t = const.tile([128, KS, DG], F32R)
        wk_t = const.tile([128, KS, DG], F32R)
        wv_t = const.tile([128, KS, DG], F32R)
        wo_t = const.tile([128, 2, HID], F32R)
        cos_t = const.tile([128, S], F32)
        sin_t = const.tile([128, S], F32)
        mask_t = const.tile([128, 256], F32)
        bq_t = const.tile([128, 2], F32)
        bk_t = const.tile([128, 2], F32)
        perm_t = const.tile([128, 4, 128], F32R)
        ones2_t = const.tile([33, 128], F32R)
        lt2_t = const.tile([33, 512], F32R)
        v1_t = const.tile([128, SB, HPG, D + 1], F32R)   # v blocks + ones col
        qr_t = const.tile([128, 2, S], F32R)   # roped q, [evens|odds] chunks
        kr_t = const.tile([128, 2, S], F32R)
        qh_t = const.tile([128, 2, S], F32R)   # head-contiguous roped q
        kh_t = const.tile([128, 2, S], F32R)
        o_t = const.tile([128, 2, S], F32R)    # attn outT (hd on partitions)

        wqr = wq.rearrange("(o p) n -> p o n", p=128)
        wkr = wk.rearrange("(o p) n -> p o n", p=128)
        wvr = wv.rearrange("(o p) n -> p o n", p=128)
        xTr = xT.rearrange("(o p) s -> p o s", p=128)


        def rope(pcs, b_t, rr_t, js):  # generator: yields mid-way
            """evens' = (e+b0)*cos - (o+b1)*sin ; odds' = (e+b0)*sin + (o+b1)*cos"""
            t1 = tmp.tile([128, 512], F32, name="t1", tag="tt")
            nc.vector.scalar_tensor_tensor(t1[:], pcs[0][:], b_t[:, 0:1],
                                           cos_t[:, js], AluOpType.add,
                                           AluOpType.mult)
            t2 = tmp.tile([128, 512], F32, name="t2", tag="tt")
            nc.vector.scalar_tensor_tensor(t2[:], pcs[1][:], b_t[:, 1:2],
                                           sin_t[:, js], AluOpType.add,
                                           AluOpType.mult)
            nc.vector.tensor_sub(rr_t[:, 0, js], t1[:], t2[:])
            yield
            t3 = tmp.tile([128, 512], F32, name="t3", tag="tt")
            nc.vector.scalar_tensor_tensor(t3[:], pcs[0][:], b_t[:, 0:1],
                                           sin_t[:, js], AluOpType.add,
                                           AluOpType.mult)
            t4 = tmp.tile([128, 512], F32, name="t4", tag="tt")
            nc.vector.scalar_tensor_tensor(t4[:], pcs[1][:], b_t[:, 1:2],
                                           cos_t[:, js], AluOpType.add,
                                           AluOpType.mult)
            nc.vector.tensor_add(rr_t[:, 1, js], t3[:], t4[:])
            yield

        mask_loaded = []
        outr = out.rearrange("(sb p) n -> sb p n", p=128)

        # early loads, in true dependency order (SP HWDGE ring is FIFO)
        for k in range(KS):
            nc.sync.dma_start(wq_t[:, k], wqr[:, k])
        nc.sync.dma_start(bq_t[:], bqp)

        def gen_attn(j):
            """Attention for sq-quarter j; yields between pipeline units so
            the emitter can weave other work into the engine streams."""
            js = bass.ts(j, 512)
            if not mask_loaded:
                mask_loaded.append(1)
                nc.gpsimd.dma_start(mask_t[:], maskT)
                for k in range(2):
                    nc.gpsimd.dma_start(
                        wo_t[:, k],
                        wo.rearrange("(o p) n -> p o n", p=128)[:, k])
            nblk = 4 * j + 4
            for cc in range(2):     # head pair (2*cc, 2*cc+1)
                pvs = [ppv.tile([D + 1, 512], F32, name="pv", tag="pv")
                       for _ in range(2)]
                for i in range(nblk):
                    # cap c0 at 256: fp32r matmuls with N<256 run at 1/4
                    # rate, so the diagonal tail block widens to N=256 and
                    # masks its leading (fully-causal-masked) 128 columns
                    c0 = min(max(0, 128 * i - 512 * j), 256)
                    n = 512 - c0
                    # both heads' scores in one 2-bank psum tile: single wide
                    # mask/exp ops; adjacent matmuls on disjoint PE row
                    # groups (base 0 / 64) overlap on HW
                    spb = psc.tile([128, 2, 512], F32, name="sp", tag="sc")
                    for a in range(2):
                        hp = slice(64 * a, 64 * a + 64)
                        nc.tensor.matmul(spb[:, a, :n],
                                         kh_t[hp, cc, bass.ts(i, 128)],
                                         qh_t[hp, cc,
                                              512 * j + c0:512 * (j + 1)],
                                         start=True, stop=True)
                    db = 128 * i - 512 * j  # diag block offset in chunk
                    if db >= 0:
                        # mask cols [mc0, mc0+128) are the triangular block;
                        # for the widened tail block the preceding 128 cols
                        # are fully masked too
                        mc0 = db - c0
                        nc.vector.tensor_add(
                            spb[:, :, 0:mc0 + 128], spb[:, :, 0:mc0 + 128],
                            mask_t[:, None, 128 - mc0:256].to_broadcast(
                                (128, 2, mc0 + 128)))
                    et = ex.tile([128, 2, 512], F32R, name="et")
                    nc.scalar.activation(et[:, :, :n], spb[:, :, :n],
                                         AF.Exp, scale=0.125)
                    yield
                    for a in range(2):
                        nc.tensor.matmul(pvs[a][:, c0:512],
                                         v1_t[:, i, 2 * cc + a, :],
                                         et[:, a, :n],
                                         start=(i == 0), stop=(i == nblk - 1))
                    yield
                # softmax denominators live in pv rows D; stack both heads'
                # rows, broadcast with one block-diag matmul, one wide recip,
                # then normalize straight into o_t (DVE handles the
                # differing partition bases)
                nc.vector.tensor_copy(lt2_t[0:1, :], pvs[0][D:D + 1, :])
                nc.vector.tensor_copy(lt2_t[32:33, :], pvs[1][D:D + 1, :])
                pvc = pvc_pool.tile([128, 512], F32, name="pvc", tag="pvc")
                for a in range(2):
                    nc.vector.tensor_copy(pvc[64 * a:64 * a + 64, :],
                                          pvs[a][0:D, :])
                bc = ps.tile([128, 512], F32, name="bc", tag="ps")
                nc.tensor.matmul(bc[:], ones2_t[:], lt2_t[:],
                                 start=True, stop=True)
                rc = nrm.tile([128, 512], F32, name="rc", bufs=1)
                nc.vector.reciprocal(rc[:], bc[:])
                yield
                for a in range(2):
                    hp2 = slice(64 * a, 64 * a + 64)
                    nc.vector.tensor_mul(o_t[hp2, cc, js],
                                         pvc[hp2, :], rc[hp2, :])
                    yield

        def gen_outproj(j):
            for sl in range(4):
                sb = 4 * j + sl
                ps0 = ps.tile([128, 512], F32, name="psC0", tag="ps")
                ps1 = ps.tile([128, 512], F32, name="psC1", tag="ps")
                for k in range(2):
                    nc.tensor.matmul(ps0[:], o_t[:, k, bass.ts(sb, 128)],
                                     wo_t[:, k, 0:512],
                                     start=(k == 0), stop=(k == 1))
                for k in range(2):
                    nc.tensor.matmul(ps1[:], o_t[:, k, bass.ts(sb, 128)],
                                     wo_t[:, k, 512:1024],
                                     start=(k == 0), stop=(k == 1))
                st = stg.tile([128, 1024], F32, name="st")
                if j == NQ - 1:   # ACT is idle at the tail
                    nc.scalar.activation(st[:, 0:512], ps0[:], AF.Copy)
                else:
                    nc.vector.tensor_copy(st[:, 0:512], ps0[:])
                nc.gpsimd.dma_start(outr[sb][:, 0:512], st[:, 0:512])
                nc.vector.tensor_copy(st[:, 512:1024], ps1[:])
                nc.gpsimd.dma_start(outr[sb][:, 512:1024], st[:, 512:1024])
                yield

        def gen_proj(qi):
            """Projections + RoPE + head-regroup for quarter qi."""
            js = bass.ts(qi, 512)
            xq = xp.tile([128, KS, 512], F32R, name="xq")
            for k in range(KS):
                # first x-quarter on the (idle-at-start) scalar ring so the
                # weight loads on the sync ring land in parallel
                eng = nc.scalar if qi == 0 else nc.sync
                eng.dma_start(xq[:, k], xTr[:, k, js])
            if qi == 0:
                nc.gpsimd.dma_start(cos_t[:], cos4)
                nc.gpsimd.dma_start(sin_t[:], sin4)
            qcs = []
            for c in range(2):
                p = ps.tile([128, 512], F32, name="psA", tag="ps")
                for k in range(KS):
                    nc.tensor.matmul(p[:], wq_t[:, k, bass.ts(c, 128)],
                                     xq[:, k, :],
                                     start=(k == 0), stop=(k == KS - 1))
                    if k == 3:
                        yield
                qcs.append(p)
                yield
            for _ in rope(qcs, bq_t, qr_t, js):
                yield
            if qi == 0:
                for k in range(KS):
                    nc.sync.dma_start(wk_t[:, k], wkr[:, k])
                nc.sync.dma_start(bk_t[:], bkp)
            kcs = []
            for c in range(2):
                p = ps.tile([128, 512], F32, name="psA", tag="ps")
                for k in range(KS):
                    nc.tensor.matmul(p[:], wk_t[:, k, bass.ts(c, 128)],
                                     xq[:, k, :],
                                     start=(k == 0), stop=(k == KS - 1))
                    if k == 3:
                        yield
                kcs.append(p)
                yield
            for _ in rope(kcs, bk_t, kr_t, js):
                yield
            if qi == 0:
                nc.gpsimd.dma_start(perm_t[:], permd)
            for rr_t, hh_t in ((qr_t, qh_t), (kr_t, kh_t)):
                # nonzero rows of perm matrix (cc, eo) live in [64cc, 64cc+64);
                # K=64 matmuls on disjoint row-strips, cc-adjacent -> overlap
                pps = [ps.tile([128, 512], F32, name="psP", tag="ps")
                       for _ in range(2)]
                for eo in range(2):
                    for cc in range(2):
                        rs = slice(64 * cc, 64 * cc + 64)
                        nc.tensor.matmul(pps[cc][:],
                                         perm_t[rs, 2 * cc + eo, :],
                                         rr_t[rs, eo, js],
                                         start=(eo == 0), stop=(eo == 1))
                for cc in range(2):
                    nc.scalar.activation(hh_t[:, cc, js], pps[cc][:], AF.Copy)
                    yield
            if qi == 0:
                for k in range(KS):
                    nc.sync.dma_start(wv_t[:, k], wvr[:, k])
                nc.gpsimd.dma_start(
                    v1_t[:, :, :, D],
                    onesd.rearrange("p (i h) -> p i h", h=HPG))
                nc.gpsimd.dma_start(ones2_t[:], ones2d)
                nc.gpsimd.dma_start(
                    lt2_t[1:32, :],
                    permd[1:32].rearrange("p a b -> p (a b)"))
            for sl in range(4):
                sb = 4 * qi + sl
                p = ps.tile([128, 512], F32, name="psAv", tag="ps")
                for k in range(KS):
                    nc.tensor.matmul(p[:, :DG],
                                     xq[:, k, bass.ts(sl, 128)],
                                     wv_t[:, k, :],
                                     start=(k == 0), stop=(k == KS - 1))
                    if k == 3:
                        yield
                nc.scalar.activation(v1_t[:, sb, :, 0:D],
                                     p[:, :DG].rearrange("p (h d) -> p h d",
                                                         d=D),
                                     AF.Copy)
                yield

        def drain(g):
            for _ in g:
                pass

        def weave(primary, *others):
            """Emit `primary` units round-robin with pieces from `others`."""
            gens = [primary] + [g for g in others if g is not None]
            while gens:
                for g in list(gens):
                    try:
                        next(g)
                    except StopIteration:
                        gens.remove(g)

        # software-pipelined emission: attention(j) woven with
        # projections(j+1) and out-proj(j-1) so each engine's static
        # instruction stream interleaves independent work
        drain(gen_proj(0))
        prev_c = None
        for j in range(NQ):
            weave(gen_attn(j),
                  gen_proj(j + 1) if j + 1 < NQ else None,
                  prev_c)
            prev_c = gen_outproj(j)
        drain(prev_c)

    nc.compile()
    return nc


_EO_IDX = None


def _eo_index():
    """Column permutation within one head group: all even components of the
    4 heads first (h-major), then all odd components."""
    global _EO_IDX
    if _EO_IDX is None:
        idx = []
        for eo in (0, 1):
            for h in range(HPG):
                idx.extend(range(64 * h + eo, 64 * h + 64, 2))
        _EO_IDX = np.asarray(idx)
    return _EO_IDX


def _perm_matrices():
    """0/1 matrices mapping roped [all-e | all-o] chunks to head-contiguous
    layout: dst chunk cc rows = [h=2cc e, h=2cc o, h=2cc+1 e, h=2cc+1 o].
    perm[:, 2*cc+eo, :]: lhsT[p_src, p_dst] for source chunk eo, dest cc."""
    perm = np.zeros((128, 4, 128), np.float32)
    for cc in range(2):
        for eo in range(2):
            m = perm[:, 2 * cc + eo, :]
            for aa in range(2):         # head-within-pair
                h = 2 * cc + aa
                for q in range(32):
                    src_row = 32 * h + q
                    dst = 64 * aa + 32 * eo + q
                    m[src_row, dst] = 1.0
    return perm


def make_in_maps(x, Wq, bq, Wk, bk, Wv, bv, Wo, bo, mask, freqs_cos, freqs_sin):
    idx = _eo_index()
    f32 = np.float32
    cosT = np.ascontiguousarray(freqs_cos.T, dtype=f32)       # (32, S)
    sinT = np.ascontiguousarray(freqs_sin.T, dtype=f32)
    cos4 = np.tile(cosT, (4, 1))                              # (128, S)
    sin4 = np.tile(sinT, (4, 1))
    m = np.asarray(mask[0, 0], dtype=f32)
    # all diagonal 128x128 blocks of a causal mask are identical; prepend a
    # fully-masked 128-col panel for the widened (N=256) tail blocks
    mT = np.concatenate([np.full((128, 128), -8e9, f32),
                         np.ascontiguousarray(m[0:128, 0:128].T * 8.0)],
                        axis=1).astype(f32)
    perm = _perm_matrices()
    ones = np.ones((128, SB * HPG), f32)
    ones2 = np.zeros((33, 128), f32)
    ones2[0, 0:64] = 1.0
    ones2[32, 64:128] = 1.0

    in_maps = []
    for core in range(NCORES):
        b, g = core // G, core % G
        cols = slice(DG * g, DG * (g + 1))
        wq_g = np.ascontiguousarray(Wq[:, cols][:, idx], dtype=f32)
        wk_g = np.ascontiguousarray(Wk[:, cols][:, idx], dtype=f32)
        wv_g = np.ascontiguousarray(Wv[:, cols], dtype=f32)
        wo_g = np.ascontiguousarray(Wo[cols, :], dtype=f32)
        bq_g = np.ascontiguousarray(
            bq[cols][idx].reshape(2, 128).T, dtype=f32)
        bk_g = np.ascontiguousarray(
            bk[cols][idx].reshape(2, 128).T, dtype=f32)
        xT_b = np.ascontiguousarray(np.asarray(x[b], dtype=f32).T)
        in_maps.append(dict(xT=xT_b, wq=wq_g, wk=wk_g, wv=wv_g, wo=wo_g,
                            bqp=bq_g, bkp=bk_g, cos4=cos4, sin4=sin4,
                            maskT=mT, permd=perm, onesd=ones, ones2d=ones2))
    return in_maps


_NC_CACHE = None
LAST_RESULTS = None


def kernel(**inputs):
    global _NC_CACHE
    if _NC_CACHE is None:
        _NC_CACHE = build_program()
    nc = _NC_CACHE

    inputs = {k: np.asarray(v) for k, v in inputs.items()}
    in_maps = make_in_maps(**inputs)
    kwargs = {}
    if os.environ.get("BASS_TRACE"):
        kwargs = dict(trace=True, trace_cores=list(range(NCORES)),
                      stitch_traces=True)
    res = run_bass_kernel_spmd(nc, in_maps, core_ids=list(range(NCORES)),
                               **kwargs)
    global LAST_RESULTS
    LAST_RESULTS = res

    out = np.zeros((B, S, HID), np.float32)
    for core in range(NCORES):
        out[core // G] += res.results[core]["out"]
    out += inputs["bo"].astype(np.float32)
    out += (inputs["bv"].astype(np.float32) @ inputs["Wo"].astype(np.float32))
    return out

